# revision 16
# baseline (speedup 1.0000x reference)
"""Trainium2 Bass kernel for nn_GAT_12232066859439.

3-layer GAT + 6-head MLP readout. Strategy:
  - GAT layers computed redundantly on all 8 cores (cheap: the N^2 attention
    collapses algebraically -- e2 has only g=N/f distinct rows -- and
    masked softmax reduces to adj * exp(s) / rowsum, so no [N,N] softmax
    materialization is needed).
  - The 402MB l1w matvec (the memory-bound bulk) is sharded 192 rows/core;
    t1 is AllGathered, l2/l3 computed redundantly; output taken from core 0.
  - fp16 data with fp32 PSUM accumulation for the big streams (1 cyc/row on
    the PE vs 4 for fp32, and half the HBM traffic).
"""
import os
import sys

sys.path.insert(0, "/opt/trn_rl_repo")

import numpy as np

import concourse.bacc as bacc
import concourse.bass as bass
import concourse.tile as tile
from concourse import mybir
from concourse.bass_utils import run_bass_kernel_spmd

F32 = mybir.dt.float32
F16 = mybir.dt.float16
U8 = mybir.dt.uint8
AF = mybir.ActivationFunctionType
ALU = mybir.AluOpType

P = 128
N = 1024
NCORES = 8
NCH = N // P  # 8 row-chunks
# (Fin, F, g) per GAT layer
LAYERS = [(512, 128, 8), (128, 64, 16), (64, 64, 16)]
RSHARD = 1536 // NCORES  # 192 l1 rows per core
KCH = 65536 // P         # 512 contraction chunks for l1
SLAB = 64                # k-chunks per A-slab DMA
D2 = 256                 # l2 contraction size

NP_GAT = np.float16

_CACHE = {}


def ts(i, n):
    return slice(i * n, (i + 1) * n)


def _build():
    nc = bacc.Bacc("TRN2", target_bir_lowering=False, debug=False,
                   num_devices=NCORES)
    DT = F16  # GAT compute dtype (PSUM accumulation is fp32 regardless)

    # ---- inputs (replicated unless noted) ----
    xT_d = nc.dram_tensor("xT", [512, N], DT, kind="ExternalInput")
    adjT_d = nc.dram_tensor("adjT", [N, N], F16, kind="ExternalInput")
    ident_d = nc.dram_tensor("ident", [P, P], F32, kind="ExternalInput")
    W_d = [nc.dram_tensor(f"W{l+1}", [LAYERS[l][0], LAYERS[l][1]], DT,
                          kind="ExternalInput") for l in range(3)]
    aF_d = [nc.dram_tensor(f"aF{l+1}", [P, LAYERS[l][1]], DT,
                           kind="ExternalInput") for l in range(3)]
    b_d = [nc.dram_tensor(f"b{l+1}", [P, LAYERS[l][1]], F32,
                          kind="ExternalInput") for l in range(3)]
    bT_d = [nc.dram_tensor(f"bT{l+1}", [P, 1], F32,
                           kind="ExternalInput") for l in range(3)]
    wsel_d = [nc.dram_tensor(f"wsel{l+1}", [N, LAYERS[l][2]], DT,
                             kind="ExternalInput") for l in range(3)]
    A_d = nc.dram_tensor("A", [P, KCH, RSHARD], F16, kind="ExternalInput")  # per-core
    l1bs_d = nc.dram_tensor("l1bs", [1, RSHARD], F32, kind="ExternalInput")  # per-core
    L2T_d = nc.dram_tensor("L2T", [D2, 6 * P], F16, kind="ExternalInput")
    l2bT_d = nc.dram_tensor("l2bT", [P, 6], F32, kind="ExternalInput")
    l3wT_d = nc.dram_tensor("l3wT", [P, 6], F32, kind="ExternalInput")
    l3b_d = nc.dram_tensor("l3b", [1, 6], F32, kind="ExternalInput")

    out_d = nc.dram_tensor("out", [6, 1], F32, kind="ExternalOutput")

    with tile.TileContext(nc) as tc:
        with tc.tile_pool(name="const", bufs=1) as const, \
             tc.tile_pool(name="work", bufs=1) as work, \
             tc.tile_pool(name="ps", bufs=1, space="PSUM") as psp, \
             tc.tile_pool(name="dram", bufs=1, space="DRAM") as dram:

            # ---- constant loads ----
            ident = const.tile([P, P], F32)
            nc.sync.dma_start(ident[:], ident_d[:])

            adjT = const.tile([P, NCH * N], DT, name="adjT_c")
            for kc in range(NCH):
                nc.sync.dma_start(adjT[:, ts(kc, N)], adjT_d[ts(kc, P), :])

            xT = const.tile([P, 4 * N], DT, name="xT_sb")
            for kc in range(4):
                nc.sync.dma_start(xT[:, ts(kc, N)], xT_d[ts(kc, P), :])

            W_sb, aF_sb, b_sb, bT_sb, wsel_sb = [], [], [], [], []
            for l, (Fin, F, g) in enumerate(LAYERS):
                nk = max(1, Fin // P)
                w = const.tile([P, nk * F], DT, name=f"W_sb{l}")
                for kc in range(nk):
                    kp = min(P, Fin)
                    nc.sync.dma_start(w[:kp, ts(kc, F)], W_d[l][ts(kc, kp), :])
                W_sb.append(w)
                af = const.tile([P, F], DT, name=f"aF_sb{l}")
                nc.sync.dma_start(af[:], aF_d[l][:])
                aF_sb.append(af)
                bb = const.tile([P, F], F32, name=f"b_sb{l}")
                nc.sync.dma_start(bb[:], b_d[l][:])
                b_sb.append(bb)
                bt = const.tile([P, 1], F32, name=f"bT_sb{l}")
                nc.sync.dma_start(bt[:], bT_d[l][:])
                bT_sb.append(bt)
                wsl = const.tile([P, NCH * g], DT, name=f"wsel_sb{l}")
                for m in range(NCH):
                    nc.sync.dma_start(wsl[:, ts(m, g)], wsel_d[l][ts(m, P), :])
                wsel_sb.append(wsl)

            l1bs = const.tile([1, RSHARD], F32)
            nc.sync.dma_start(l1bs[:], l1bs_d[:])
            L2T = const.tile([P, 2 * 6 * P], F16, name="L2T_sb")
            for k in range(2):
                nc.sync.dma_start(L2T[:, ts(k, 6 * P)], L2T_d[ts(k, P), :])
            l2bT = const.tile([P, 6], F32)
            nc.sync.dma_start(l2bT[:], l2bT_d[:])
            l3wT = const.tile([P, 6], F32)
            nc.sync.dma_start(l3wT[:], l3wT_d[:])
            l3b = const.tile([1, 6], F32)
            nc.sync.dma_start(l3b[:], l3b_d[:])

            ones_row = const.tile([1, P], F32, name="ones_row")
            nc.vector.memset(ones_row[:], 1.0)

            warm_in = dram.tile([1, 8], F32, name="warm_in")
            warm_out = dram.tile([NCORES, 8], F32, name="warm_out",
                                 addr_space="Shared")
            nc.vector.memset(wz := work.tile([1, 8], F32, name="wz"), 0.0)
            nc.sync.dma_start(warm_in[:], wz[:])
            nc.gpsimd.collective_compute(
                "AllGather", ALU.bypass,
                replica_groups=[list(range(NCORES))],
                ins=[warm_in.opt()], outs=[warm_out.opt()])

            h3v = dram.tile([512, P], F32, name="h3v")

            # ---- GAT layers ----
            prev_hT = None  # [F_prev, N] sbuf tile for layers 2,3
            for l, (Fin, F, g) in enumerate(LAYERS):
                nk = max(1, Fin // P)
                kp = min(P, Fin)

                haug = work.tile([P, NCH * (F + 1)], DT, name=f"haug{l}",
                                 tag="haug", bufs=2)
                e1 = work.tile([P, NCH], F32, name=f"e1_{l}", tag="e1", bufs=2)

                for m in range(NCH):
                    hp = psp.tile([P, F], F32, name=f"hp{l}_{m}", tag="hproj",
                                  bufs=1)
                    for kc in range(nk):
                        if l == 0:
                            lhsT = xT[:, kc * N + m * P: kc * N + (m + 1) * P]
                        else:
                            lhsT = prev_hT[:kp, ts(m, P)]
                        nc.tensor.matmul(hp[:], lhsT, W_sb[l][:kp, ts(kc, F)],
                                         start=(kc == 0), stop=(kc == nk - 1))
                    col = m * (F + 1)
                    nc.vector.tensor_copy(haug[:, col: col + F], hp[:])
                    nc.vector.memset(haug[:, col + F: col + F + 1], 1.0)
                haug_v = haug.rearrange("p (m f) -> p m f", f=F + 1)[:, :, 0:F]
                tmpa = work.tile([P, NCH * F], F32, name=f"e1t{l}",
                                 tag="e1tmp", bufs=2)
                nc.vector.tensor_mul(
                    tmpa.rearrange("p (m f) -> p m f", f=F), haug_v,
                    aF_sb[l][:].broadcast_to([P, F, NCH]).rearrange(
                        "p f m -> p m f"))
                nc.vector.reduce_sum(e1[:], tmpa.rearrange(
                    "p (m f) -> p m f", f=F), axis=mybir.AxisListType.X)

                # E2T[v, u] = sum_i wsel[i, u] * hproj[i, v]
                e2ps = psp.tile([F, g], F32, name=f"e2ps{l}", tag="small",
                                bufs=1)
                for m in range(NCH):
                    col = m * (F + 1)
                    nc.tensor.matmul(e2ps[:], haug[:, col: col + F],
                                     wsel_sb[l][:, ts(m, g)],
                                     start=(m == 0), stop=(m == NCH - 1))
                e2T = work.tile([P, g], F32, name=f"e2T{l}", tag="e2T", bufs=2)
                nc.vector.tensor_copy(e2T[:F, :], e2ps[:])
                if F == 64:
                    nc.sync.dma_start(e2T[64:128, :], e2T[0:64, :])

                # e_all[:, kc*g + u] = exp(lrelu(e1[:, kc] + E2T[:, u]))
                s_scr = work.tile([P, NCH * g], F32, name=f"sscr{l}",
                                  tag="sscr", bufs=2)
                nc.vector.tensor_add(
                    s_scr.rearrange("p (k u) -> p k u", u=g),
                    e1[:].broadcast_to([P, NCH, g]),
                    e2T[:, 0:g].broadcast_to([P, g, NCH]).rearrange(
                        "p u k -> p k u"))
                nc.vector.scalar_tensor_tensor(s_scr[:], s_scr[:], 0.2,
                                               s_scr[:], ALU.mult, ALU.max)
                e_all = work.tile([P, NCH * g], DT, name=f"eall{l}",
                                  tag="eall", bufs=2)
                nc.scalar.activation(e_all[:], s_scr[:], AF.Exp)

                # attention: numer[i, :] = sum_j adj[i,j] E_{u(i)}[j] haug[j, :]
                adjT_v = adjT.rearrange("p (k j) -> p k j", j=N)
                e_all_v = e_all.rearrange("p (k u) -> p k u", u=g)
                if l < 2:
                    new_hT = work.tile([P, N], DT, name=f"hT{l}",
                                       tag="hT", bufs=2)
                for m in range(NCH):
                    w = work.tile([P, NCH * P], DT, name=f"wun{l}_{m}",
                                  tag="wun", bufs=3)
                    if F == P:
                        nc.vector.tensor_mul(
                            w.rearrange("p (k j) -> p k j", j=P),
                            adjT_v[:, :, m * P: (m + 1) * P],
                            e_all[:, m: NCH * g: g].broadcast_to(
                                [P, NCH, P]))
                    else:
                        nc.vector.tensor_mul(
                            w.rearrange("p (k uu v) -> p k uu v", uu=2, v=64),
                            adjT_v[:, :, m * P: (m + 1) * P].rearrange(
                                "p k (uu v) -> p k uu v", v=64),
                            e_all_v[:, :, 2 * m: 2 * m + 2].broadcast_to(
                                [P, NCH, 2, 64]))
                    nps = psp.tile([P, F + 1], F32, name=f"nps{l}_{m}",
                                   tag="numer", bufs=3)
                    for kc in range(NCH):
                        nc.tensor.matmul(nps[:], w[:, ts(kc, P)],
                                         haug[:, kc * (F + 1): (kc + 1) * (F + 1)],
                                         start=(kc == 0), stop=(kc == NCH - 1))
                    rd = work.tile([P, 1], F32, name=f"rd{l}_{m}", tag="rd",
                                   bufs=2)
                    nc.vector.reciprocal(rd[:], nps[:, F: F + 1])
                    y = work.tile([P, F], F32, name=f"y{l}_{m}", tag="y",
                                  bufs=2)
                    nc.vector.tensor_scalar(y[:], nps[:, 0:F], rd[:], 0.0,
                                            ALU.mult, ALU.max)
                    hn = work.tile([P, F], F32, name=f"hn{l}_{m}", tag="hn",
                                   bufs=3)
                    nc.vector.tensor_add(hn[:], y[:], b_sb[l][:])
                    nc.vector.tensor_scalar_max(hn[:], hn[:], 0.0)
                    if l < 2:
                        tp = psp.tile([F, P], F32, name=f"tp{l}_{m}",
                                      tag="tp", bufs=2)
                        nc.tensor.transpose(tp[:], hn[:], ident[:])
                        nc.vector.tensor_copy(new_hT[:F, ts(m, P)], tp[:])
                    else:
                        nc.sync.dma_start(h3v[ts(m, 64), :], hn[:])
                if l < 2:
                    prev_hT = new_hT

            # ---- MLP head ----
            # xf_sb[p, c] = xf[c*128 + p]
            xf = work.tile([P, KCH], F16, name="xf_sb")
            for t in range(4):
                v = work.tile([P, P], F32, name=f"xfin{t}", tag="xfin", bufs=2)
                nc.sync.dma_start(v[:], h3v[ts(t, P), :])
                tp = psp.tile([P, P], F32, name=f"xtp{t}", tag="tp", bufs=2)
                nc.tensor.transpose(tp[:], v[:], ident[:])
                nc.vector.tensor_copy(xf[:, ts(t, P)], tp[:])

            t1ps = psp.tile([2, 2 * RSHARD], F32, name="t1ps", tag="t1ps",
                            bufs=1)
            nslab = KCH // SLAB
            npair = KCH // 2
            for si in range(nslab):
                a_sb = work.tile([P, SLAB * RSHARD], F16, name=f"aslab{si}",
                                 tag="aslab", bufs=3)
                nc.scalar.dma_start(a_sb[:], A_d[:, ts(si, SLAB), :])
                for pp in range(SLAB // 2):
                    p_ = si * (SLAB // 2) + pp
                    nc.tensor.matmul(t1ps[:], xf[:, 2 * p_: 2 * p_ + 2],
                                     a_sb[:, ts(pp, 2 * RSHARD)],
                                     start=(p_ == 0), stop=(p_ == npair - 1))
            t1c = work.tile([2, 2 * RSHARD], F32, name="t1c")
            nc.vector.tensor_copy(t1c[:], t1ps[:])
            t1b = work.tile([1, RSHARD], F32, name="t1b")
            nc.sync.dma_start(t1b[:], t1c[1:2, RSHARD:2 * RSHARD])
            t1h = work.tile([1, RSHARD], F32, name="t1h")
            nc.vector.tensor_add(t1h[:], t1c[0:1, 0:RSHARD], t1b[:])
            t1a = work.tile([1, RSHARD], F32, name="t1a")
            nc.vector.tensor_add(t1a[:], t1h[:], l1bs[:])
            nc.vector.tensor_scalar_max(t1a[:], t1a[:], 0.0)

            ag_in = dram.tile([1, RSHARD], F32, name="ag_in")
            ag_out = dram.tile([NCORES, RSHARD], F32, name="ag_out",
                               addr_space="Shared")
            nc.sync.dma_start(ag_in[:], t1a[:])
            nc.gpsimd.collective_compute(
                "AllGather", ALU.bypass,
                replica_groups=[list(range(NCORES))],
                ins=[ag_in.opt()], outs=[ag_out.opt()])

            t1n = work.tile([12, P], F32, name="t1n")
            nc.sync.dma_start(
                t1n[:], ag_out.rearrange("a b -> (a b)").rearrange(
                    "(a b) -> a b", b=P))
            t1tp = psp.tile([P, 12], F32, name="t1tp", tag="small", bufs=1)
            nc.tensor.transpose(t1tp[:], t1n[:], ident[0:12, 0:12])
            t1T = work.tile([P, 12], F16, name="t1T")
            nc.vector.tensor_copy(t1T[:], t1tp[:])

            t2 = work.tile([P, 6], F32, name="t2_sb")
            for h in range(6):
                t2ps = psp.tile([P, 1], F32, name=f"t2ps{h}", tag="small",
                                bufs=1)
                for k in range(2):
                    nc.tensor.matmul(t2ps[:],
                                     L2T[:, k * 6 * P + h * P: k * 6 * P + (h + 1) * P],
                                     t1T[:, 2 * h + k: 2 * h + k + 1],
                                     start=(k == 0), stop=(k == 1))
                nc.scalar.activation(t2[:, h: h + 1], t2ps[:], AF.Sigmoid,
                                     bias=l2bT[:, h: h + 1])

            ones = const.tile([P, 1], F32, name="ones_col")
            nc.vector.memset(ones[:], 1.0)
            p3 = work.tile([P, 6], F32, name="p3")
            nc.vector.tensor_mul(p3[:], t2[:], l3wT[:])
            ops_ = psp.tile([1, 6], F32, name="outps", tag="small", bufs=1)
            nc.tensor.matmul(ops_[:], ones[:], p3[:], start=True, stop=True)
            osb = work.tile([1, 6], F32, name="osb")
            nc.vector.tensor_add(osb[:], ops_[:], l3b[:])
            nc.sync.dma_start(out_d[:], osb[:])

    nc.compile()
    return nc


def _prep_inputs(inputs):
    x = np.asarray(inputs["x"], dtype=np.float32)
    adj = np.asarray(inputs["adj"])
    common = {
        "xT": np.ascontiguousarray(x.T.astype(NP_GAT)),
        "adjT": np.ascontiguousarray((adj.T > 0).astype(np.float16)),
        "ident": np.eye(P, dtype=np.float32),
    }
    for l, (Fin, F, g) in enumerate(LAYERS):
        a = np.asarray(inputs[f"a{l+1}"], dtype=np.float32)
        common[f"W{l+1}"] = np.ascontiguousarray(
            np.asarray(inputs[f"W{l+1}"], dtype=np.float32).astype(NP_GAT))
        common[f"aF{l+1}"] = np.ascontiguousarray(
            np.broadcast_to(a[:F], (P, F)).astype(NP_GAT))
        bv = np.asarray(inputs[f"b{l+1}"], dtype=np.float32)
        common[f"b{l+1}"] = np.ascontiguousarray(np.broadcast_to(bv, (P, F)))
        btc = np.zeros((P, 1), dtype=np.float32)
        btc[:F, 0] = bv
        common[f"bT{l+1}"] = btc
        aS = a[F:]
        i = np.arange(N)
        wsel = np.zeros((N, g), dtype=np.float32)
        wsel[i, i % g] = aS[i // g]
        common[f"wsel{l+1}"] = wsel.astype(NP_GAT)

    l2w = np.asarray(inputs["l2w"], dtype=np.float32)  # [6, 128, 256]
    common["L2T"] = np.ascontiguousarray(
        l2w.transpose(2, 0, 1).reshape(D2, 6 * P).astype(np.float16))
    common["l2bT"] = np.ascontiguousarray(
        np.asarray(inputs["l2b"], dtype=np.float32).T)       # [128, 6]
    common["l3wT"] = np.ascontiguousarray(
        np.asarray(inputs["l3w"], dtype=np.float32)[:, 0, :].T)  # [128, 6]
    common["l3b"] = np.ascontiguousarray(
        np.asarray(inputs["l3b"], dtype=np.float32).reshape(1, 6))

    l1w_flat = np.asarray(inputs["l1w"], dtype=np.float32).reshape(1536, 65536)
    l1b_flat = np.asarray(inputs["l1b"], dtype=np.float32).reshape(1536)
    in_maps = []
    for c in range(NCORES):
        rows = l1w_flat[c * RSHARD:(c + 1) * RSHARD]       # [192, 65536]
        A = np.ascontiguousarray(
            rows.T.reshape(KCH, P, RSHARD).transpose(1, 0, 2).astype(np.float16))
        m = dict(common)
        m["A"] = A
        m["l1bs"] = np.ascontiguousarray(
            l1b_flat[c * RSHARD:(c + 1) * RSHARD].reshape(1, RSHARD))
        in_maps.append(m)
    return in_maps


def _ensure_ntff_hook():
    """Register the axon NTFF profile hook (the image's antenv lacks
    axon_hooks; supply it in sys.modules so bass_utils can trace)."""
    try:
        import types

        import antenv
        if "antenv.axon_hooks" not in sys.modules:
            mod = types.ModuleType("antenv.axon_hooks")
            mod._hook = None

            def _set(h, _m=mod):
                _m._hook = h

            def _get(_m=mod):
                return _m._hook

            mod.set_axon_ntff_profile_hook = _set
            mod.get_axon_ntff_profile_hook = _get
            sys.modules["antenv.axon_hooks"] = mod
            antenv.axon_hooks = mod
        from antenv.axon_hooks import (get_axon_ntff_profile_hook,
                                       set_axon_ntff_profile_hook)
        if get_axon_ntff_profile_hook() is None:
            from trn_agent_boot.trn_boot import _ntff_profile_via_ctypes
            set_axon_ntff_profile_hook(
                _ntff_profile_via_ctypes("/opt/axon/libaxon_pjrt.so"))
        return True
    except Exception as e:  # pragma: no cover - profiling is best-effort
        print(f"ntff hook unavailable: {e}", file=sys.stderr)
        return False


def kernel(**inputs) -> np.ndarray:
    if "nc" not in _CACHE:
        _CACHE["nc"] = _build()
    nc = _CACHE["nc"]
    in_maps = _prep_inputs(inputs)
    trace = bool(int(os.environ.get("BASS_KERNEL_TRACE", "0")))
    if trace:
        trace = _ensure_ntff_hook()
    res = run_bass_kernel_spmd(nc, in_maps, list(range(NCORES)), trace=trace)
    _CACHE["last_results"] = res
    return np.asarray(res.results[0]["out"]).reshape(6, 1)
